# revision 5
# baseline (speedup 1.0000x reference)
"""MACE edge-message block on 8 Trainium2 NeuronCores (Bass/Tile).

Strategy (data-parallel over edges, hinted):
  - 100k edges padded to 102400, sharded 12800/core across 8 cores.
  - Node features replicated; host pre-transposes them to a bf16 table with
    column order [s | vx | vy | vz] so that `dma_gather(transpose=True)`
    delivers feature-major tiles [128 ch, C edges] directly (no on-chip
    transposes).
  - Whole pipeline is feature-major: radial MLP, tensor-product weights,
    up-projection of gathered sender features, CG tensor product (elementwise,
    with per-edge SH scalars partition-broadcast via a 0-stride DMA), and the
    final per-irrep linear, all as [K<=128, M<=128] x [K, C] matmuls.
  - All e3nn normalization constants / path weights / SiLU norm are folded
    into the weights on the host.
Output is written as a transposed [512, EP] f32 tensor per core and
re-assembled on the host.
"""

import numpy as np
import ml_dtypes
from contextlib import ExitStack

N_NODES = 20000
N_EDGES = 100000
MUL = 128
R = 8
H = 64
NCORES = 8
ESH = N_EDGES // NCORES          # 12500 real edges per core
C = 512                          # edge chunk (free dim)
EP = 12800                       # padded edges per core (25 * 512)
NCHUNK = EP // C
SILU_NORM = 1.6790390826
INV_SQRT3 = 1.0 / np.sqrt(3.0)
PW_0E = np.sqrt(0.5)
PW_1O = np.sqrt(1.5)
BF16 = ml_dtypes.bfloat16

_CACHE = {}


def _build_program():
    import concourse.bass as bass
    import concourse.tile as tile
    from concourse import bacc, mybir

    bf = mybir.dt.bfloat16
    f32 = mybir.dt.float32
    i16 = mybir.dt.int16
    Silu = mybir.ActivationFunctionType.Silu

    nc = bacc.Bacc(
        "TRN2",
        target_bir_lowering=False,
        debug=False,
        num_devices=NCORES,
        num_swdge_queues=4,
    )

    nft = nc.dram_tensor("nft", [N_NODES, 512], bf, kind="ExternalInput")
    eft = nc.dram_tensor("eft", [R, EP], bf, kind="ExternalInput")
    eat = nc.dram_tensor("eat", [4, EP], bf, kind="ExternalInput")
    idx = nc.dram_tensor("idx", [128, EP // 16], i16, kind="ExternalInput")
    w0 = nc.dram_tensor("w0", [R, H], bf, kind="ExternalInput")
    w1 = nc.dram_tensor("w1", [H, H], bf, kind="ExternalInput")
    w2 = nc.dram_tensor("w2", [H, H], bf, kind="ExternalInput")
    w3 = nc.dram_tensor("w3", [H, 512], bf, kind="ExternalInput")
    wup = nc.dram_tensor("wup", [128, 256], bf, kind="ExternalInput")
    wout = nc.dram_tensor("wout", [128, 512], bf, kind="ExternalInput")
    outt = nc.dram_tensor("outt", [512, EP], f32, kind="ExternalOutput")

    with tile.TileContext(nc) as tc, ExitStack() as ctx:
        const = ctx.enter_context(tc.tile_pool(name="const", bufs=1))

        def load_const(dram, shape, dt_, name):
            t = const.tile(shape, dt_, name=name, tag=name)
            nc.sync.dma_start(t[:], dram[:])
            return t

        w0s = load_const(w0, [R, H], bf, "w0s")
        w1s = load_const(w1, [H, H], bf, "w1s")
        w2s = load_const(w2, [H, H], bf, "w2s")
        w3s = load_const(w3, [H, 512], bf, "w3s")
        wups = load_const(wup, [128, 256], bf, "wups")
        wouts = load_const(wout, [128, 512], bf, "wouts")
        idxs = load_const(idx, [128, EP // 16], i16, "idxs")

        gp = ctx.enter_context(tc.tile_pool(name="gp", bufs=3))
        bp = ctx.enter_context(tc.tile_pool(name="bp", bufs=3))
        ep = ctx.enter_context(tc.tile_pool(name="ep", bufs=3))
        sb = ctx.enter_context(tc.tile_pool(name="sb", bufs=2))
        ob = ctx.enter_context(tc.tile_pool(name="ob", bufs=2))
        ph = ctx.enter_context(tc.tile_pool(name="ph", bufs=1, space="PSUM"))
        pm = ctx.enter_context(tc.tile_pool(name="pm", bufs=5, space="PSUM"))
        po = ctx.enter_context(tc.tile_pool(name="po", bufs=2, space="PSUM"))

        for j in range(NCHUNK):
            c0 = j * C

            # ---- inputs for this chunk ----
            G = gp.tile([128, 4, C], bf)
            nc.gpsimd.dma_gather(
                G[:], nft[:], idxs[:, c0 // 16:(c0 + C) // 16],
                C, C, 512, transpose=True, queue_num=j % 4,
            )
            ef = ep.tile([R, C], bf)
            nc.sync.dma_start(ef[:], eft[:, c0:c0 + C])
            # per-edge SH scalars, partition-broadcast: B[p, k, e] = eat[k, c0+e]
            B = bp.tile([128, 4, C], bf)
            nc.sync.dma_start(B[:], bass.AP(eat, c0, [[0, 128], [EP, 4], [1, C]]))

            # ---- radial MLP ----
            h1p = ph.tile([H, C], f32, tag="hp")
            nc.tensor.matmul(h1p[:], w0s[:], ef[:], start=True, stop=True)
            h1 = sb.tile([H, C], bf, tag="h1")
            nc.scalar.activation(h1[:], h1p[:], Silu)
            h2p = ph.tile([H, C], f32, tag="hp")
            nc.tensor.matmul(h2p[:], w1s[:], h1[:], start=True, stop=True)
            h2 = sb.tile([H, C], bf, tag="h2")
            nc.scalar.activation(h2[:], h2p[:], Silu)
            h3p = ph.tile([H, C], f32, tag="hp")
            nc.tensor.matmul(h3p[:], w2s[:], h2[:], start=True, stop=True)
            h3 = sb.tile([H, C], bf, tag="h3")
            nc.scalar.activation(h3[:], h3p[:], Silu)
            # h3 pre-scaled by sh0 for the blocks whose path carries sh0
            h3s = sb.tile([H, C], bf, tag="h3s")
            nc.vector.tensor_mul(h3s[:], h3[:], B[0:H, 0, :])

            # ---- tensor-product weights (sh0 pre-folded into blocks 00/10) ----
            w00p = pm.tile([128, C], f32, tag="mm")
            nc.tensor.matmul(w00p[:], w3s[:, 0:128], h3s[:], start=True, stop=True)
            w01p = pm.tile([128, C], f32, tag="mm")
            nc.tensor.matmul(w01p[:], w3s[:, 128:256], h3[:], start=True, stop=True)
            w10p = pm.tile([128, C], f32, tag="mm")
            nc.tensor.matmul(w10p[:], w3s[:, 256:384], h3s[:], start=True, stop=True)
            w11p = pm.tile([128, C], f32, tag="mm")
            nc.tensor.matmul(w11p[:], w3s[:, 384:512], h3[:], start=True, stop=True)
            t00 = sb.tile([128, C], bf, tag="t00")
            nc.scalar.copy(t00[:], w00p[:])
            t01 = sb.tile([128, C], bf, tag="t01")
            nc.scalar.copy(t01[:], w01p[:])
            t10 = sb.tile([128, C], bf, tag="t10")
            nc.scalar.copy(t10[:], w10p[:])
            t11 = sb.tile([128, C], bf, tag="t11")
            nc.vector.tensor_copy(t11[:], w11p[:])

            # ---- up-projection of gathered sender features ----
            ssp = pm.tile([128, C], f32, tag="mm")
            nc.tensor.matmul(ssp[:], wups[:, 0:128], G[:, 0, :], start=True, stop=True)
            ss = sb.tile([128, C], bf, tag="ss")
            nc.scalar.copy(ss[:], ssp[:])
            vsp = [pm.tile([128, C], f32, tag="mm", name=f"vsp{i}") for i in range(3)]
            for i in range(3):
                nc.tensor.matmul(
                    vsp[i][:], wups[:, 128:256], G[:, 1 + i, :], start=True, stop=True
                )

            # ---- CG tensor product (elementwise, feature-major) ----
            a0 = sb.tile([128, C], bf, tag="a0")
            nc.vector.tensor_mul(a0[:], t00[:], ss[:])
            m01 = sb.tile([128, C], bf, tag="m01")
            nc.vector.tensor_mul(m01[:], t01[:], ss[:])
            a1 = [sb.tile([128, C], bf, tag=f"a1_{i}", name=f"a1_{i}") for i in range(3)]
            for i in range(3):
                nc.vector.tensor_mul(a1[i][:], m01[:], B[:, 1 + i, :])
            q = [sb.tile([128, C], bf, tag=f"q{i}", name=f"q{i}") for i in range(3)]
            for i in range(3):
                nc.vector.tensor_mul(q[i][:], t10[:], vsp[i][:])
            # d = sum_i vs_i * sh1_i, via raw gathered v and one extra matmul
            dt_ = sb.tile([128, 3, C], bf, tag="dt")
            nc.vector.tensor_mul(dt_[:], G[:, 1:4, :], B[:, 1:4, :])
            dr = sb.tile([128, C], bf, tag="dr")
            nc.vector.tensor_add(dr[:], dt_[:, 0, :], dt_[:, 1, :])
            dr2 = sb.tile([128, C], bf, tag="dr2")
            nc.vector.tensor_add(dr2[:], dr[:], dt_[:, 2, :])
            dp = pm.tile([128, C], f32, tag="mm")
            nc.tensor.matmul(dp[:], wups[:, 128:256], dr2[:], start=True, stop=True)
            b0 = sb.tile([128, C], bf, tag="b0")
            nc.vector.tensor_mul(b0[:], t11[:], dp[:])

            # ---- output linears (K split 128+128, PSUM accumulate) ----
            osp = po.tile([128, C], f32, tag="oo")
            nc.tensor.matmul(osp[:], wouts[:, 0:128], a0[:], start=True, stop=False)
            nc.tensor.matmul(osp[:], wouts[:, 128:256], b0[:], start=False, stop=True)
            os_sb = ob.tile([128, C], f32, tag="os_sb")
            nc.scalar.copy(os_sb[:], osp[:])
            nc.sync.dma_start(outt[0:128, c0:c0 + C], os_sb[:])
            for i in range(3):
                ovp = po.tile([128, C], f32, tag="oo", name=f"ovp{i}")
                nc.tensor.matmul(
                    ovp[:], wouts[:, 256:384], a1[i][:], start=True, stop=False
                )
                nc.tensor.matmul(
                    ovp[:], wouts[:, 384:512], q[i][:], start=False, stop=True
                )
                ov_sb = ob.tile([128, C], f32, tag=f"ov_sb{i}", name=f"ov_sb{i}")
                if i == 0:
                    nc.scalar.copy(ov_sb[:], ovp[:])
                else:
                    nc.vector.tensor_copy(ov_sb[:], ovp[:])
                nc.sync.dma_start(
                    outt[128 * (1 + i):128 * (2 + i), c0:c0 + C], ov_sb[:]
                )

    nc.compile()
    return nc


def _get_program():
    if "nc" not in _CACHE:
        _CACHE["nc"] = _build_program()
    return _CACHE["nc"]


def _prep_static(node_feats, W_up_s, W_up_v, mlp_w0, mlp_w1, mlp_w2, mlp_w3,
                 W_out_s, W_out_v):
    """Host-side weight/node-table prep (shared across cores)."""
    nf = np.asarray(node_feats, np.float32)
    s = nf[:, :MUL]
    v = nf[:, MUL:].reshape(N_NODES, MUL, 3)
    nft = np.concatenate([s, v[:, :, 0], v[:, :, 1], v[:, :, 2]], axis=1)

    w0 = np.asarray(mlp_w0, np.float32) / np.sqrt(R)
    w1 = np.asarray(mlp_w1, np.float32) / np.sqrt(H) * SILU_NORM
    w2 = np.asarray(mlp_w2, np.float32) / np.sqrt(H) * SILU_NORM
    w3 = np.asarray(mlp_w3, np.float32) / np.sqrt(H) * SILU_NORM

    wup = np.concatenate(
        [np.asarray(W_up_s, np.float32), np.asarray(W_up_v, np.float32)], axis=1
    ) / np.sqrt(MUL)

    wos = np.asarray(W_out_s, np.float32) / np.sqrt(2 * MUL)
    wov = np.asarray(W_out_v, np.float32) / np.sqrt(2 * MUL)
    wos_top = wos[:MUL] * PW_0E
    wos_bot = wos[MUL:] * (PW_0E * INV_SQRT3)
    wov_sc = wov * (PW_1O * INV_SQRT3)
    wout = np.concatenate(
        [wos_top, wos_bot, wov_sc[:MUL], wov_sc[MUL:]], axis=1
    )

    return dict(
        nft=np.ascontiguousarray(nft).astype(BF16),
        w0=np.ascontiguousarray(w0).astype(BF16),
        w1=np.ascontiguousarray(w1).astype(BF16),
        w2=np.ascontiguousarray(w2).astype(BF16),
        w3=np.ascontiguousarray(w3).astype(BF16),
        wup=np.ascontiguousarray(wup).astype(BF16),
        wout=np.ascontiguousarray(wout).astype(BF16),
    )


def _prep_core(k, sender, edge_attrs, edge_feats):
    lo, hi = k * ESH, (k + 1) * ESH
    ef = np.zeros((EP, R), np.float32)
    ef[:ESH] = edge_feats[lo:hi]
    ea = np.zeros((EP, 4), np.float32)
    ea[:ESH] = edge_attrs[lo:hi]
    snd = np.zeros((EP,), np.int16)
    snd[:ESH] = sender[lo:hi].astype(np.int16)
    wrapped = snd.reshape(EP // 16, 16).T          # idx i -> [i%16, i//16]
    idx16 = np.ascontiguousarray(np.tile(wrapped, (8, 1)))  # replicate to 128 parts
    return dict(
        eft=np.ascontiguousarray(ef.T).astype(BF16),
        eat=np.ascontiguousarray(ea.T).astype(BF16),
        idx=idx16,
    )


def kernel(node_feats, edge_attrs, edge_feats, edge_index,
           W_up_s, W_up_v, mlp_w0, mlp_w1, mlp_w2, mlp_w3,
           W_out_s, W_out_v, _want_results=False, _trace=False):
    from concourse.bass_utils import run_bass_kernel_spmd

    nc = _get_program()

    static = _prep_static(node_feats, W_up_s, W_up_v, mlp_w0, mlp_w1, mlp_w2,
                          mlp_w3, W_out_s, W_out_v)
    sender = np.asarray(edge_index)[0]
    ea = np.asarray(edge_attrs, np.float32)
    ef = np.asarray(edge_feats, np.float32)

    in_maps = []
    for k in range(NCORES):
        m = dict(static)
        m.update(_prep_core(k, sender, ea, ef))
        in_maps.append(m)

    res = run_bass_kernel_spmd(
        nc, in_maps, core_ids=list(range(NCORES)), trace=_trace
    )

    out = np.empty((N_EDGES, 4 * MUL), np.float32)
    for k in range(NCORES):
        ot = res.results[k]["outt"][:, :ESH]           # [512, ESH]
        lo, hi = k * ESH, (k + 1) * ESH
        out[lo:hi, :MUL] = ot[:MUL].T
        out[lo:hi, MUL:] = (
            ot[MUL:].reshape(3, MUL, ESH).transpose(2, 1, 0).reshape(ESH, 3 * MUL)
        )
    if _want_results:
        return out, res
    return out


# revision 7
# speedup vs baseline: 10.4395x; 10.4395x over previous
"""MACE edge-message block on 8 Trainium2 NeuronCores (Bass/Tile).

Strategy (data-parallel over edges, hinted):
  - 100k edges padded to 102400, sharded 12800/core across 8 cores.
  - Node features replicated; host pre-transposes them to a bf16 table with
    column order [s | vx | vy | vz] so that `dma_gather(transpose=True)`
    delivers feature-major tiles [128 ch, C edges] directly (no on-chip
    transposes).
  - Whole pipeline is feature-major: radial MLP, tensor-product weights,
    up-projection of gathered sender features, CG tensor product (elementwise,
    with per-edge SH scalars partition-broadcast via a 0-stride DMA), and the
    final per-irrep linear, all as [K<=128, M<=128] x [K, C] matmuls.
  - All e3nn normalization constants / path weights / SiLU norm are folded
    into the weights on the host.
Output is written as a transposed [512, EP] f32 tensor per core and
re-assembled on the host.
"""

import numpy as np
import ml_dtypes
from contextlib import ExitStack

N_NODES = 20000
N_EDGES = 100000
MUL = 128
R = 8
H = 64
NCORES = 8
ESH = N_EDGES // NCORES          # 12500 real edges per core
C = 512                          # edge chunk (free dim)
EP = 12800                       # padded edges per core (25 * 512)
NCHUNK = EP // C
SILU_NORM = 1.6790390826
INV_SQRT3 = 1.0 / np.sqrt(3.0)
PW_0E = np.sqrt(0.5)
PW_1O = np.sqrt(1.5)
BF16 = ml_dtypes.bfloat16

OUT_BF16 = True

_CACHE = {}


def _build_program(reps=1):
    import concourse.bass as bass
    import concourse.tile as tile
    from concourse import bacc, mybir

    bf = mybir.dt.bfloat16
    f32 = mybir.dt.float32
    i16 = mybir.dt.int16
    Silu = mybir.ActivationFunctionType.Silu

    nc = bacc.Bacc(
        "TRN2",
        target_bir_lowering=False,
        debug=False,
        num_devices=NCORES,
        num_swdge_queues=4,
    )

    nft = nc.dram_tensor("nft", [N_NODES, 512], bf, kind="ExternalInput")
    eft = nc.dram_tensor("eft", [R, EP], bf, kind="ExternalInput")
    eat = nc.dram_tensor("eat", [4, EP], bf, kind="ExternalInput")
    idx = nc.dram_tensor("idx", [128, EP // 16], i16, kind="ExternalInput")
    w0 = nc.dram_tensor("w0", [R, H], bf, kind="ExternalInput")
    w1 = nc.dram_tensor("w1", [H, H], bf, kind="ExternalInput")
    w2 = nc.dram_tensor("w2", [H, H], bf, kind="ExternalInput")
    w3 = nc.dram_tensor("w3", [H, 512], bf, kind="ExternalInput")
    wup = nc.dram_tensor("wup", [128, 256], bf, kind="ExternalInput")
    wout = nc.dram_tensor("wout", [128, 512], bf, kind="ExternalInput")
    odt = bf if OUT_BF16 else f32
    outt = nc.dram_tensor("outt", [512, EP], odt, kind="ExternalOutput")

    with tile.TileContext(nc) as tc, ExitStack() as ctx:
        const = ctx.enter_context(tc.tile_pool(name="const", bufs=1))

        def load_const(dram, shape, dt_, name):
            t = const.tile(shape, dt_, name=name, tag=name)
            nc.sync.dma_start(t[:], dram[:])
            return t

        w0s = load_const(w0, [R, H], bf, "w0s")
        w1s = load_const(w1, [H, H], bf, "w1s")
        w2s = load_const(w2, [H, H], bf, "w2s")
        w3s = load_const(w3, [H, 512], bf, "w3s")
        wups = load_const(wup, [128, 256], bf, "wups")
        wouts = load_const(wout, [128, 512], bf, "wouts")
        idxs = load_const(idx, [128, EP // 16], i16, "idxs")

        gp = ctx.enter_context(tc.tile_pool(name="gp", bufs=4))
        bp = ctx.enter_context(tc.tile_pool(name="bp", bufs=4))
        ep = ctx.enter_context(tc.tile_pool(name="ep", bufs=4))
        sb = ctx.enter_context(tc.tile_pool(name="sb", bufs=3))
        ob = ctx.enter_context(tc.tile_pool(name="ob", bufs=3))
        ph = ctx.enter_context(tc.tile_pool(name="ph", bufs=1, space="PSUM"))
        pm = ctx.enter_context(tc.tile_pool(name="pm", bufs=5, space="PSUM"))
        po = ctx.enter_context(tc.tile_pool(name="po", bufs=2, space="PSUM"))

        rep_cm = tc.For_i(0, reps, 1) if reps > 1 else None
        if rep_cm is not None:
            rep_cm.__enter__()
        for j in range(NCHUNK):
            c0 = j * C

            # ---- inputs for this chunk ----
            G = gp.tile([128, 4, C], bf)
            nc.gpsimd.dma_gather(
                G[:], nft[:], idxs[:, c0 // 16:(c0 + C) // 16],
                C, C, 512, transpose=True, queue_num=j % 4,
            )
            ef = ep.tile([R, C], bf)
            nc.scalar.dma_start(ef[:], eft[:, c0:c0 + C])
            # per-edge SH scalars, partition-broadcast: B[p, k, e] = eat[k, c0+e]
            B = bp.tile([128, 4, C], bf)
            nc.scalar.dma_start(B[:], bass.AP(eat, c0, [[0, 128], [EP, 4], [1, C]]))

            # ---- radial MLP ----
            h1p = ph.tile([H, C], f32, tag="hp")
            nc.tensor.matmul(h1p[:], w0s[:], ef[:], start=True, stop=True)
            h1 = sb.tile([H, C], bf, tag="h1")
            nc.scalar.activation(h1[:], h1p[:], Silu)
            h2p = ph.tile([H, C], f32, tag="hp")
            nc.tensor.matmul(h2p[:], w1s[:], h1[:], start=True, stop=True)
            h2 = sb.tile([H, C], bf, tag="h2")
            nc.scalar.activation(h2[:], h2p[:], Silu)
            h3p = ph.tile([H, C], f32, tag="hp")
            nc.tensor.matmul(h3p[:], w2s[:], h2[:], start=True, stop=True)
            h3 = sb.tile([H, C], bf, tag="h3")
            nc.scalar.activation(h3[:], h3p[:], Silu)
            # h3 pre-scaled by sh0 for the blocks whose path carries sh0
            h3s = sb.tile([H, C], bf, tag="h3s")
            nc.vector.tensor_mul(h3s[:], h3[:], B[0:H, 0, :])

            # ---- tensor-product weights (sh0 pre-folded into blocks 00/10) ----
            w00p = pm.tile([128, C], f32, tag="mm")
            nc.tensor.matmul(w00p[:], w3s[:, 0:128], h3s[:], start=True, stop=True)
            w01p = pm.tile([128, C], f32, tag="mm")
            nc.tensor.matmul(w01p[:], w3s[:, 128:256], h3[:], start=True, stop=True)
            w10p = pm.tile([128, C], f32, tag="mm")
            nc.tensor.matmul(w10p[:], w3s[:, 256:384], h3s[:], start=True, stop=True)
            w11p = pm.tile([128, C], f32, tag="mm")
            nc.tensor.matmul(w11p[:], w3s[:, 384:512], h3[:], start=True, stop=True)
            t00 = sb.tile([128, C], bf, tag="t00")
            nc.scalar.copy(t00[:], w00p[:])
            t01 = sb.tile([128, C], bf, tag="t01")
            nc.scalar.copy(t01[:], w01p[:])
            t10 = sb.tile([128, C], bf, tag="t10")
            nc.scalar.copy(t10[:], w10p[:])
            t11 = sb.tile([128, C], bf, tag="t11")
            nc.vector.tensor_copy(t11[:], w11p[:])

            # ---- up-projection of gathered sender features ----
            ssp = pm.tile([128, C], f32, tag="mm")
            nc.tensor.matmul(ssp[:], wups[:, 0:128], G[:, 0, :], start=True, stop=True)
            ss = sb.tile([128, C], bf, tag="ss")
            nc.scalar.copy(ss[:], ssp[:])
            vsp = [pm.tile([128, C], f32, tag="mm", name=f"vsp{i}") for i in range(3)]
            for i in range(3):
                nc.tensor.matmul(
                    vsp[i][:], wups[:, 128:256], G[:, 1 + i, :], start=True, stop=True
                )

            # ---- CG tensor product (elementwise, feature-major) ----
            a0 = sb.tile([128, C], bf, tag="a0")
            nc.vector.tensor_mul(a0[:], t00[:], ss[:])
            m01 = sb.tile([128, C], bf, tag="m01")
            nc.vector.tensor_mul(m01[:], t01[:], ss[:])
            a1p = sb.tile([128, 3, C], bf, tag="a1p", name="a1p")
            m01_ap = m01[:]
            m01_rep = bass.AP(m01_ap.tensor, m01_ap.offset,
                              [list(m01_ap.ap[0]), [0, 3], list(m01_ap.ap[1])])
            nc.vector.tensor_mul(a1p[:], m01_rep, B[:, 1:4, :])
            a1 = [a1p[:, i, :] for i in range(3)]
            q = [sb.tile([128, C], bf, tag=f"q{i}", name=f"q{i}") for i in range(3)]
            for i in range(3):
                nc.vector.tensor_mul(q[i][:], t10[:], vsp[i][:])
            # d = sum_i vs_i * sh1_i, via raw gathered v and one extra matmul
            dt_ = sb.tile([128, 3, C], bf, tag="dt")
            nc.vector.tensor_mul(dt_[:], G[:, 1:4, :], B[:, 1:4, :])
            dr = sb.tile([128, C], bf, tag="dr")
            nc.gpsimd.tensor_add(dr[:], dt_[:, 0, :], dt_[:, 1, :])
            dr2 = sb.tile([128, C], bf, tag="dr2")
            nc.gpsimd.tensor_add(dr2[:], dr[:], dt_[:, 2, :])
            dp = pm.tile([128, C], f32, tag="mm")
            nc.tensor.matmul(dp[:], wups[:, 128:256], dr2[:], start=True, stop=True)
            b0 = sb.tile([128, C], bf, tag="b0")
            nc.vector.tensor_mul(b0[:], t11[:], dp[:])

            # ---- output linears (K split 128+128, PSUM accumulate) ----
            osp = po.tile([128, C], f32, tag="oo")
            nc.tensor.matmul(osp[:], wouts[:, 0:128], a0[:], start=True, stop=False)
            nc.tensor.matmul(osp[:], wouts[:, 128:256], b0[:], start=False, stop=True)
            os_sb = ob.tile([128, C], odt, tag="os_sb")
            nc.scalar.copy(os_sb[:], osp[:])
            nc.sync.dma_start(outt[0:128, c0:c0 + C], os_sb[:])
            for i in range(3):
                ovp = po.tile([128, C], f32, tag="oo", name=f"ovp{i}")
                nc.tensor.matmul(
                    ovp[:], wouts[:, 256:384], a1[i][:], start=True, stop=False
                )
                nc.tensor.matmul(
                    ovp[:], wouts[:, 384:512], q[i][:], start=False, stop=True
                )
                ov_sb = ob.tile([128, C], odt, tag=f"ov_sb{i}", name=f"ov_sb{i}")
                if i == 0:
                    nc.scalar.copy(ov_sb[:], ovp[:])
                else:
                    nc.vector.tensor_copy(ov_sb[:], ovp[:])
                nc.sync.dma_start(
                    outt[128 * (1 + i):128 * (2 + i), c0:c0 + C], ov_sb[:]
                )
        if rep_cm is not None:
            rep_cm.__exit__(None, None, None)

    nc.compile()
    return nc


def _get_program():
    if "nc" not in _CACHE:
        _CACHE["nc"] = _build_program()
    return _CACHE["nc"]


def _prep_static(node_feats, W_up_s, W_up_v, mlp_w0, mlp_w1, mlp_w2, mlp_w3,
                 W_out_s, W_out_v):
    """Host-side weight/node-table prep (shared across cores)."""
    nf = np.asarray(node_feats, np.float32)
    s = nf[:, :MUL]
    v = nf[:, MUL:].reshape(N_NODES, MUL, 3)
    nft = np.concatenate([s, v[:, :, 0], v[:, :, 1], v[:, :, 2]], axis=1)

    w0 = np.asarray(mlp_w0, np.float32) / np.sqrt(R)
    w1 = np.asarray(mlp_w1, np.float32) / np.sqrt(H) * SILU_NORM
    w2 = np.asarray(mlp_w2, np.float32) / np.sqrt(H) * SILU_NORM
    w3 = np.asarray(mlp_w3, np.float32) / np.sqrt(H) * SILU_NORM

    wup = np.concatenate(
        [np.asarray(W_up_s, np.float32), np.asarray(W_up_v, np.float32)], axis=1
    ) / np.sqrt(MUL)

    wos = np.asarray(W_out_s, np.float32) / np.sqrt(2 * MUL)
    wov = np.asarray(W_out_v, np.float32) / np.sqrt(2 * MUL)
    wos_top = wos[:MUL] * PW_0E
    wos_bot = wos[MUL:] * (PW_0E * INV_SQRT3)
    wov_sc = wov * (PW_1O * INV_SQRT3)
    wout = np.concatenate(
        [wos_top, wos_bot, wov_sc[:MUL], wov_sc[MUL:]], axis=1
    )

    return dict(
        nft=np.ascontiguousarray(nft).astype(BF16),
        w0=np.ascontiguousarray(w0).astype(BF16),
        w1=np.ascontiguousarray(w1).astype(BF16),
        w2=np.ascontiguousarray(w2).astype(BF16),
        w3=np.ascontiguousarray(w3).astype(BF16),
        wup=np.ascontiguousarray(wup).astype(BF16),
        wout=np.ascontiguousarray(wout).astype(BF16),
    )


def _prep_core(k, sender, edge_attrs, edge_feats):
    lo, hi = k * ESH, (k + 1) * ESH
    ef = np.zeros((EP, R), np.float32)
    ef[:ESH] = edge_feats[lo:hi]
    ea = np.zeros((EP, 4), np.float32)
    ea[:ESH] = edge_attrs[lo:hi]
    snd = np.zeros((EP,), np.int16)
    snd[:ESH] = sender[lo:hi].astype(np.int16)
    wrapped = snd.reshape(EP // 16, 16).T          # idx i -> [i%16, i//16]
    idx16 = np.ascontiguousarray(np.tile(wrapped, (8, 1)))  # replicate to 128 parts
    return dict(
        eft=np.ascontiguousarray(ef.T).astype(BF16),
        eat=np.ascontiguousarray(ea.T).astype(BF16),
        idx=idx16,
    )


def kernel(node_feats, edge_attrs, edge_feats, edge_index,
           W_up_s, W_up_v, mlp_w0, mlp_w1, mlp_w2, mlp_w3,
           W_out_s, W_out_v, _want_results=False, _trace=False):
    from concourse.bass_utils import run_bass_kernel_spmd

    nc = _get_program()

    static = _prep_static(node_feats, W_up_s, W_up_v, mlp_w0, mlp_w1, mlp_w2,
                          mlp_w3, W_out_s, W_out_v)
    sender = np.asarray(edge_index)[0]
    ea = np.asarray(edge_attrs, np.float32)
    ef = np.asarray(edge_feats, np.float32)

    in_maps = []
    for k in range(NCORES):
        m = dict(static)
        m.update(_prep_core(k, sender, ea, ef))
        in_maps.append(m)

    res = run_bass_kernel_spmd(
        nc, in_maps, core_ids=list(range(NCORES)), trace=_trace
    )

    out = np.empty((N_EDGES, 4 * MUL), np.float32)
    for k in range(NCORES):
        ot = np.asarray(res.results[k]["outt"], np.float32)[:, :ESH]
        lo, hi = k * ESH, (k + 1) * ESH
        out[lo:hi, :MUL] = ot[:MUL].T
        out[lo:hi, MUL:] = (
            ot[MUL:].reshape(3, MUL, ESH).transpose(2, 1, 0).reshape(ESH, 3 * MUL)
        )
    if _want_results:
        return out, res
    return out


# revision 12
# speedup vs baseline: 22.4333x; 2.1489x over previous
"""MACE edge-message block on 8 Trainium2 NeuronCores (Bass/Tile).

Strategy (data-parallel over edges, hinted):
  - 100k edges padded to 102400, sharded 12800/core across 8 cores.
  - Node features replicated; host pre-transposes them to a bf16 table with
    column order [s | vx | vy | vz] so that `dma_gather(transpose=True)`
    delivers feature-major tiles [128 ch, C edges] directly (no on-chip
    transposes).
  - Whole pipeline is feature-major: radial MLP, tensor-product weights,
    up-projection of gathered sender features, CG tensor product (elementwise,
    with per-edge SH scalars partition-broadcast via a 0-stride DMA), and the
    final per-irrep linear, all as [K<=128, M<=128] x [K, C] matmuls.
  - All e3nn normalization constants / path weights / SiLU norm are folded
    into the weights on the host.
Output is written as a transposed [512, EP] f32 tensor per core and
re-assembled on the host.
"""

import numpy as np
import ml_dtypes
from contextlib import ExitStack

N_NODES = 20000
N_EDGES = 100000
MUL = 128
R = 8
H = 64
NCORES = 8
ESH = N_EDGES // NCORES          # 12500 real edges per core
C = 512                          # edge chunk (free dim)
EP = 12800                       # padded edges per core (25 * 512)
NCHUNK = EP // C
SILU_NORM = 1.6790390826
INV_SQRT3 = 1.0 / np.sqrt(3.0)
PW_0E = np.sqrt(0.5)
PW_1O = np.sqrt(1.5)
BF16 = ml_dtypes.bfloat16

OUT_BF16 = True
GP_ADDS = False      # dr adds on gpsimd (else DVE)
SCALAR_DMA = True    # chunk input DMAs on scalar HWDGE ring (else sync)
A1_PACK = False      # single packed a1 TT (else 3 TTs)
PROBE = ""           # timing ablation: "", "store", "gather", "bcast", "tp"
PM_MERGE = True      # MLP h psums share the pm pool
DIRECT_PSUM = False  # a0/m01 read w00p/w01p straight from PSUM

_CACHE = {}


def _build_program(reps=1):
    import concourse.bass as bass
    import concourse.tile as tile
    from concourse import bacc, mybir

    bf = mybir.dt.bfloat16
    f32 = mybir.dt.float32
    i16 = mybir.dt.int16
    Silu = mybir.ActivationFunctionType.Silu

    nc = bacc.Bacc(
        "TRN2",
        target_bir_lowering=False,
        debug=False,
        num_devices=NCORES,
        num_swdge_queues=4,
    )

    nft = nc.dram_tensor("nft", [N_NODES, 512], bf, kind="ExternalInput")
    eft = nc.dram_tensor("eft", [R, EP], bf, kind="ExternalInput")
    eat = nc.dram_tensor("eat", [4, EP], bf, kind="ExternalInput")
    idx = nc.dram_tensor("idx", [128, EP // 16], i16, kind="ExternalInput")
    w0 = nc.dram_tensor("w0", [R, H], bf, kind="ExternalInput")
    w1 = nc.dram_tensor("w1", [H, H], bf, kind="ExternalInput")
    w2 = nc.dram_tensor("w2", [H, H], bf, kind="ExternalInput")
    w3 = nc.dram_tensor("w3", [H, 512], bf, kind="ExternalInput")
    wup = nc.dram_tensor("wup", [128, 256], bf, kind="ExternalInput")
    wout = nc.dram_tensor("wout", [128, 512], bf, kind="ExternalInput")
    odt = bf if OUT_BF16 else f32
    outt = nc.dram_tensor("outt", [512, EP], odt, kind="ExternalOutput")

    with tile.TileContext(nc) as tc, ExitStack() as ctx:
        const = ctx.enter_context(tc.tile_pool(name="const", bufs=1))

        def load_const(dram, shape, dt_, name):
            t = const.tile(shape, dt_, name=name, tag=name)
            nc.sync.dma_start(t[:], dram[:])
            return t

        w0s = load_const(w0, [R, H], bf, "w0s")
        w1s = load_const(w1, [H, H], bf, "w1s")
        w2s = load_const(w2, [H, H], bf, "w2s")
        w3s = load_const(w3, [H, 512], bf, "w3s")
        wups = load_const(wup, [128, 256], bf, "wups")
        wouts = load_const(wout, [128, 512], bf, "wouts")
        idxs = load_const(idx, [128, EP // 16], i16, "idxs")

        gp = ctx.enter_context(tc.tile_pool(name="gp", bufs=4))
        bp = ctx.enter_context(tc.tile_pool(name="bp", bufs=4))
        ep = ctx.enter_context(tc.tile_pool(name="ep", bufs=4))
        sb = ctx.enter_context(tc.tile_pool(name="sb", bufs=3))
        ob = ctx.enter_context(tc.tile_pool(name="ob", bufs=3))
        if PM_MERGE:
            pm = ctx.enter_context(tc.tile_pool(name="pm", bufs=6, space="PSUM"))
            ph = pm
            htag = "mm"
        else:
            ph = ctx.enter_context(tc.tile_pool(name="ph", bufs=1, space="PSUM"))
            pm = ctx.enter_context(tc.tile_pool(name="pm", bufs=5, space="PSUM"))
            htag = "hp"
        po = ctx.enter_context(tc.tile_pool(name="po", bufs=2, space="PSUM"))

        rep_cm = tc.For_i(0, reps, 1) if reps > 1 else None
        if rep_cm is not None:
            rep_cm.__enter__()
        for j in range(NCHUNK):
            c0 = j * C

            # ---- inputs for this chunk ----
            if PROBE == "gather":
                if j == 0:
                    G = gp.tile([128, 4, C], bf, tag="Gfix", name="Gfix")
                    nc.gpsimd.dma_gather(
                        G[:], nft[:], idxs[:, 0:C // 16],
                        C, C, 512, transpose=True, queue_num=0,
                    )
                    _Gfix = G
                else:
                    G = _Gfix
            else:
                G = gp.tile([128, 4, C], bf)
                nc.gpsimd.dma_gather(
                    G[:], nft[:], idxs[:, c0 // 16:(c0 + C) // 16],
                    C, C, 512, transpose=True, queue_num=j % 4,
                )
            ef = ep.tile([R, C], bf)
            ineng = nc.scalar if SCALAR_DMA else nc.sync
            ineng.dma_start(ef[:], eft[:, c0:c0 + C])
            # per-edge SH scalars, partition-broadcast: B[p, k, e] = eat[k, c0+e]
            if PROBE == "bcast":
                if j == 0:
                    B = bp.tile([128, 4, C], bf, tag="Bfix", name="Bfix")
                    ineng.dma_start(B[:], bass.AP(eat, 0, [[0, 128], [EP, 4], [1, C]]))
                    _Bfix = B
                else:
                    B = _Bfix
            else:
                B = bp.tile([128, 4, C], bf)
                ineng.dma_start(B[:], bass.AP(eat, c0, [[0, 128], [EP, 4], [1, C]]))

            # ---- radial MLP ----
            h1p = ph.tile([H, C], f32, tag=htag)
            nc.tensor.matmul(h1p[:], w0s[:], ef[:], start=True, stop=True)
            h1 = sb.tile([H, C], bf, tag="h1")
            nc.scalar.activation(h1[:], h1p[:], Silu)
            h2p = ph.tile([H, C], f32, tag=htag)
            nc.tensor.matmul(h2p[:], w1s[:], h1[:], start=True, stop=True)
            h2 = sb.tile([H, C], bf, tag="h2")
            nc.scalar.activation(h2[:], h2p[:], Silu)
            h3p = ph.tile([H, C], f32, tag=htag)
            nc.tensor.matmul(h3p[:], w2s[:], h2[:], start=True, stop=True)
            h3 = sb.tile([H, C], bf, tag="h3")
            nc.scalar.activation(h3[:], h3p[:], Silu)
            # h3 pre-scaled by sh0 for the blocks whose path carries sh0
            h3s = sb.tile([H, C], bf, tag="h3s")
            nc.vector.tensor_mul(h3s[:], h3[:], B[0:H, 0, :])

            # ---- tensor-product weights (sh0 pre-folded into blocks 00/10) ----
            w00p = pm.tile([128, C], f32, tag="mm")
            nc.tensor.matmul(w00p[:], w3s[:, 0:128], h3s[:], start=True, stop=True)
            w01p = pm.tile([128, C], f32, tag="mm")
            nc.tensor.matmul(w01p[:], w3s[:, 128:256], h3[:], start=True, stop=True)
            w10p = pm.tile([128, C], f32, tag="mm")
            nc.tensor.matmul(w10p[:], w3s[:, 256:384], h3s[:], start=True, stop=True)
            w11p = pm.tile([128, C], f32, tag="mm")
            nc.tensor.matmul(w11p[:], w3s[:, 384:512], h3[:], start=True, stop=True)
            if not DIRECT_PSUM:
                t00 = sb.tile([128, C], bf, tag="t00")
                nc.scalar.copy(t00[:], w00p[:])
                t01 = sb.tile([128, C], bf, tag="t01")
                nc.scalar.copy(t01[:], w01p[:])
            t10 = sb.tile([128, C], bf, tag="t10")
            nc.scalar.copy(t10[:], w10p[:])
            t11 = sb.tile([128, C], bf, tag="t11")
            nc.vector.tensor_copy(t11[:], w11p[:])

            # ---- up-projection of gathered sender features ----
            ssp = pm.tile([128, C], f32, tag="mm")
            nc.tensor.matmul(ssp[:], wups[:, 0:128], G[:, 0, :], start=True, stop=True)
            ss = sb.tile([128, C], bf, tag="ss")
            nc.scalar.copy(ss[:], ssp[:])
            vsp = [pm.tile([128, C], f32, tag="mm", name=f"vsp{i}") for i in range(3)]
            for i in range(3):
                nc.tensor.matmul(
                    vsp[i][:], wups[:, 128:256], G[:, 1 + i, :], start=True, stop=True
                )

            # ---- CG tensor product (elementwise, feature-major) ----
            if PROBE == "tp":
                a0, qv = t10, [t10, t10, t10]
                a1 = [t10[:], t10[:], t10[:]]
            else:
                a0 = sb.tile([128, C], bf, tag="a0")
                nc.vector.tensor_mul(a0[:], w00p[:] if DIRECT_PSUM else t00[:], ss[:])
            if PROBE != "tp":
                m01 = sb.tile([128, C], bf, tag="m01")
                nc.vector.tensor_mul(m01[:], w01p[:] if DIRECT_PSUM else t01[:], ss[:])
            if PROBE == "tp":
                pass
            elif A1_PACK:
                a1p = sb.tile([128, 3, C], bf, tag="a1p", name="a1p")
                m01_ap = m01[:]
                m01_rep = bass.AP(m01_ap.tensor, m01_ap.offset,
                                  [list(m01_ap.ap[0]), [0, 3], list(m01_ap.ap[1])])
                nc.vector.tensor_mul(a1p[:], m01_rep, B[:, 1:4, :])
                a1 = [a1p[:, i, :] for i in range(3)]
            else:
                a1t = [sb.tile([128, C], bf, tag=f"a1_{i}", name=f"a1_{i}") for i in range(3)]
                for i in range(3):
                    nc.vector.tensor_mul(a1t[i][:], m01[:], B[:, 1 + i, :])
                a1 = [t[:] for t in a1t]
            if PROBE == "tp":
                q = qv
            else:
                q = [sb.tile([128, C], bf, tag=f"q{i}", name=f"q{i}") for i in range(3)]
                for i in range(3):
                    nc.vector.tensor_mul(q[i][:], t10[:], vsp[i][:])
            # d = sum_i vs_i * sh1_i, via raw gathered v and one extra matmul
            if PROBE != "tp":
                dt_ = sb.tile([128, 3, C], bf, tag="dt")
                nc.vector.tensor_mul(dt_[:], G[:, 1:4, :], B[:, 1:4, :])
            if PROBE == "tp":
                b0 = t10
            else:
                addeng = nc.gpsimd if GP_ADDS else nc.vector
                dr = sb.tile([128, C], bf, tag="dr")
                addeng.tensor_add(dr[:], dt_[:, 0, :], dt_[:, 1, :])
                dr2 = sb.tile([128, C], bf, tag="dr2")
                addeng.tensor_add(dr2[:], dr[:], dt_[:, 2, :])
                dp = pm.tile([128, C], f32, tag="mm")
                nc.tensor.matmul(dp[:], wups[:, 128:256], dr2[:], start=True, stop=True)
                b0 = sb.tile([128, C], bf, tag="b0")
                nc.vector.tensor_mul(b0[:], t11[:], dp[:])

            # ---- output linears (K split 128+128, PSUM accumulate) ----
            osp = po.tile([128, C], f32, tag="oo")
            nc.tensor.matmul(osp[:], wouts[:, 0:128], a0[:], start=True, stop=False)
            nc.tensor.matmul(osp[:], wouts[:, 128:256], b0[:], start=False, stop=True)
            os_sb = ob.tile([128, C], odt, tag="os_sb")
            nc.scalar.copy(os_sb[:], osp[:])
            if PROBE == "store":
                nc.sync.dma_start(outt[0:128, c0:c0 + 8], os_sb[:, 0:8])
            else:
                nc.sync.dma_start(outt[0:128, c0:c0 + C], os_sb[:])
            for i in range(3):
                ovp = po.tile([128, C], f32, tag="oo", name=f"ovp{i}")
                nc.tensor.matmul(
                    ovp[:], wouts[:, 256:384], a1[i][:], start=True, stop=False
                )
                nc.tensor.matmul(
                    ovp[:], wouts[:, 384:512], q[i][:], start=False, stop=True
                )
                ov_sb = ob.tile([128, C], odt, tag=f"ov_sb{i}", name=f"ov_sb{i}")
                nc.vector.tensor_copy(ov_sb[:], ovp[:])
                if PROBE == "store":
                    nc.sync.dma_start(
                        outt[128 * (1 + i):128 * (2 + i), c0:c0 + 8], ov_sb[:, 0:8]
                    )
                else:
                    nc.sync.dma_start(
                        outt[128 * (1 + i):128 * (2 + i), c0:c0 + C], ov_sb[:]
                    )
        if rep_cm is not None:
            rep_cm.__exit__(None, None, None)

    nc.compile()
    return nc


def _get_program():
    if "nc" not in _CACHE:
        _CACHE["nc"] = _build_program()
    return _CACHE["nc"]


def _prep_static(node_feats, W_up_s, W_up_v, mlp_w0, mlp_w1, mlp_w2, mlp_w3,
                 W_out_s, W_out_v):
    """Host-side weight/node-table prep (shared across cores)."""
    nf = np.asarray(node_feats, np.float32)
    s = nf[:, :MUL]
    v = nf[:, MUL:].reshape(N_NODES, MUL, 3)
    nft = np.concatenate([s, v[:, :, 0], v[:, :, 1], v[:, :, 2]], axis=1)

    w0 = np.asarray(mlp_w0, np.float32) / np.sqrt(R)
    w1 = np.asarray(mlp_w1, np.float32) / np.sqrt(H) * SILU_NORM
    w2 = np.asarray(mlp_w2, np.float32) / np.sqrt(H) * SILU_NORM
    w3 = np.asarray(mlp_w3, np.float32) / np.sqrt(H) * SILU_NORM

    wup = np.concatenate(
        [np.asarray(W_up_s, np.float32), np.asarray(W_up_v, np.float32)], axis=1
    ) / np.sqrt(MUL)

    wos = np.asarray(W_out_s, np.float32) / np.sqrt(2 * MUL)
    wov = np.asarray(W_out_v, np.float32) / np.sqrt(2 * MUL)
    wos_top = wos[:MUL] * PW_0E
    wos_bot = wos[MUL:] * (PW_0E * INV_SQRT3)
    wov_sc = wov * (PW_1O * INV_SQRT3)
    wout = np.concatenate(
        [wos_top, wos_bot, wov_sc[:MUL], wov_sc[MUL:]], axis=1
    )

    return dict(
        nft=np.ascontiguousarray(nft).astype(BF16),
        w0=np.ascontiguousarray(w0).astype(BF16),
        w1=np.ascontiguousarray(w1).astype(BF16),
        w2=np.ascontiguousarray(w2).astype(BF16),
        w3=np.ascontiguousarray(w3).astype(BF16),
        wup=np.ascontiguousarray(wup).astype(BF16),
        wout=np.ascontiguousarray(wout).astype(BF16),
    )


def _prep_core(k, sender, edge_attrs, edge_feats):
    lo, hi = k * ESH, (k + 1) * ESH
    ef = np.zeros((EP, R), np.float32)
    ef[:ESH] = edge_feats[lo:hi]
    ea = np.zeros((EP, 4), np.float32)
    ea[:ESH] = edge_attrs[lo:hi]
    snd = np.zeros((EP,), np.int16)
    snd[:ESH] = sender[lo:hi].astype(np.int16)
    wrapped = snd.reshape(EP // 16, 16).T          # idx i -> [i%16, i//16]
    idx16 = np.ascontiguousarray(np.tile(wrapped, (8, 1)))  # replicate to 128 parts
    return dict(
        eft=np.ascontiguousarray(ef.T).astype(BF16),
        eat=np.ascontiguousarray(ea.T).astype(BF16),
        idx=idx16,
    )


def kernel(node_feats, edge_attrs, edge_feats, edge_index,
           W_up_s, W_up_v, mlp_w0, mlp_w1, mlp_w2, mlp_w3,
           W_out_s, W_out_v, _want_results=False, _trace=False):
    from concourse.bass_utils import run_bass_kernel_spmd

    nc = _get_program()

    static = _prep_static(node_feats, W_up_s, W_up_v, mlp_w0, mlp_w1, mlp_w2,
                          mlp_w3, W_out_s, W_out_v)
    sender = np.asarray(edge_index)[0]
    ea = np.asarray(edge_attrs, np.float32)
    ef = np.asarray(edge_feats, np.float32)

    in_maps = []
    for k in range(NCORES):
        m = dict(static)
        m.update(_prep_core(k, sender, ea, ef))
        in_maps.append(m)

    res = run_bass_kernel_spmd(
        nc, in_maps, core_ids=list(range(NCORES)), trace=_trace
    )

    out = np.empty((N_EDGES, 4 * MUL), np.float32)
    for k in range(NCORES):
        ot = np.asarray(res.results[k]["outt"], np.float32)[:, :ESH]
        lo, hi = k * ESH, (k + 1) * ESH
        out[lo:hi, :MUL] = ot[:MUL].T
        out[lo:hi, MUL:] = (
            ot[MUL:].reshape(3, MUL, ESH).transpose(2, 1, 0).reshape(ESH, 3 * MUL)
        )
    if _want_results:
        return out, res
    return out
